# revision 3
# baseline (speedup 1.0000x reference)
"""Trainium2 Bass kernel for a chain of 20 radial flows on [8388608, 4] fp32.

Strategy: pure data parallel over 8 NeuronCores. Each core gets B/8 samples,
host-transposed to [4, S] so components sit on separate SBUF partitions
(partition 32*c + b holds component c of sample-block b).

Per flow k (sequential chain):
  d   = v + delta_{k-1}            (delta folded as per-partition ACT bias)
  sq  = d*d                        ACT Square
  r2  = sum_c sq[c]                PE matmul with 0/1 stationary (float32r),
                                   output *packed* onto all 128 partitions
  r   = sqrt(r2)                   ACT
  w   = r + alpha_k                GPSIMD tensor_scalar (per-partition AP)
  y   = 1/w                        DVE custom reciprocal (~51 ULP)
  m   = beta_k*y + 1               GPSIMD tensor_scalar
  m_b = broadcast m to comps       PE matmul with 0/1 stationary -> PSUM
  v  <- (v + delta_{k-1}) * m_b    DVE scalar_tensor_tensor (fused)
Final: out = v + x0[19].
"""

import sys

if "/opt/trn_rl_repo" not in sys.path:
    sys.path.insert(0, "/opt/trn_rl_repo")

from contextlib import ExitStack

import numpy as np

import concourse.bass as bass
import concourse.tile as tile
from concourse import bacc, mybir

F32 = mybir.dt.float32
F32R = mybir.dt.float32r

BATCH = 8388608
DIM = 4
N_FLOWS = 20
N_CORES = 8
S = BATCH // N_CORES          # samples per core
NB = 32                       # sample-blocks (per quadrant of partitions)
FC = 2048                     # chunk free-dim (columns) per tile
NT = S // NB // FC            # chunks per core
PK = FC // 4                  # packed (per-sample) free-dim per chunk

_CACHE = {}


def _build_program(n_flows=N_FLOWS, nt=NT):
    nc = bacc.Bacc("TRN2", target_bir_lowering=False, debug=False,
                   num_devices=N_CORES)
    s = nt * NB * FC
    xt = nc.dram_tensor("xt", [DIM, s], F32, kind="ExternalInput").ap()
    wm = nc.dram_tensor("wm", [8, 128, 128], F32R, kind="ExternalInput").ap()
    pr = nc.dram_tensor("pr", [128, 3 * N_FLOWS + 1], F32,
                        kind="ExternalInput").ap()
    ot = nc.dram_tensor("ot", [DIM, s], F32, kind="ExternalOutput").ap()

    # DRAM view: partition (c*32+b) <- comp c, block b; free (t, f)
    xt_r = xt.rearrange("c (b t f) -> (c b) t f", b=NB, f=FC)
    ot_r = ot.rearrange("c (b t f) -> (c b) t f", b=NB, f=FC)

    AL = mybir.AluOpType
    ACT = mybir.ActivationFunctionType

    def col(i):  # per-partition scalar AP from the params tile
        return pr_t[:, i:i + 1]

    with tile.TileContext(nc) as tc, ExitStack() as ctx:
        singles = ctx.enter_context(tc.tile_pool(name="singles", bufs=1))
        state = ctx.enter_context(tc.tile_pool(name="state", bufs=1))
        sq_pool = ctx.enter_context(tc.tile_pool(name="sq", bufs=2))
        pk_pool = ctx.enter_context(tc.tile_pool(name="pk", bufs=3))
        r2_pool = ctx.enter_context(
            tc.tile_pool(name="r2", bufs=2, space="PSUM"))
        mb_pool = ctx.enter_context(
            tc.tile_pool(name="mb", bufs=2, space="PSUM"))

        pr_t = singles.tile([128, 3 * N_FLOWS + 1], F32)
        nc.sync.dma_start(pr_t[:], pr[:])
        wm_t = singles.tile([128, 8, 128], F32R)
        nc.sync.dma_start(wm_t[:], wm.rearrange("j k m -> k j m"))

        v = state.tile([128, nt * FC], F32)
        for t in range(nt):
            nc.sync.dma_start(v[:, bass.ts(t, FC)], xt_r[:, t, :])

        for k in range(n_flows):
            c_dprev, c_alpha, c_beta = k, N_FLOWS + 1 + k, 2 * N_FLOWS + 1 + k
            for t in range(nt):
                vchunk = v[:, bass.ts(t, FC)]
                sq = sq_pool.tile([128, FC], F32R)
                nc.scalar.activation(sq[:], vchunk, ACT.Square,
                                     bias=col(c_dprev), scale=1.0)
                r2 = r2_pool.tile([128, PK], F32)
                for j in range(4):
                    nc.tensor.matmul(
                        out=r2[:],
                        lhsT=wm_t[:, j, :],
                        rhs=sq[:, bass.ts(j, PK)],
                        start=(j == 0), stop=(j == 3))
                r = pk_pool.tile([128, PK], F32, tag="r")
                nc.scalar.activation(r[:], r2[:], ACT.Sqrt)
                w = pk_pool.tile([128, PK], F32, tag="w")
                nc.gpsimd.tensor_scalar(w[:], r[:], col(c_alpha), None, AL.add)
                y = pk_pool.tile([128, PK], F32, tag="y")
                nc.vector.reciprocal_approx_fast(y[:], w[:])
                m = pk_pool.tile([128, PK], F32R, tag="m")
                nc.gpsimd.tensor_scalar(m[:], y[:], col(c_beta), 1.0,
                                        AL.mult, AL.add)
                for h in range(2):  # two PSUM half-tiles so bcast/stt overlap
                    mb = mb_pool.tile([128, FC // 2], F32)
                    for jj in range(2):
                        j = 2 * h + jj
                        nc.tensor.matmul(
                            out=mb[:, bass.ts(jj, PK)],
                            lhsT=wm_t[:, 4 + j, :],
                            rhs=m[:], start=True, stop=True)
                    vh = v[:, t * FC + h * (FC // 2):
                           t * FC + (h + 1) * (FC // 2)]
                    nc.vector.scalar_tensor_tensor(
                        out=vh, in0=vh, scalar=col(c_dprev), in1=mb[:],
                        op0=AL.add, op1=AL.mult)

        for t in range(nt):
            vchunk = v[:, bass.ts(t, FC)]
            nc.vector.tensor_scalar(vchunk, vchunk, col(N_FLOWS), None, AL.add)
            nc.sync.dma_start(ot_r[:, t, :], vchunk)

    nc.compile()
    return nc


def _host_params(x0s, alpha_primes, beta_primes, n_flows=N_FLOWS):
    x0s = np.asarray(x0s, np.float32)
    sp_a = np.logaddexp(np.float32(0.0), np.asarray(alpha_primes, np.float32))
    sp_b = np.logaddexp(np.float32(0.0), np.asarray(beta_primes, np.float32))
    alpha = sp_a.astype(np.float32)
    beta = (-alpha + sp_b).astype(np.float32)

    # params: [128, 3*N_FLOWS+1]: dprev[k], final, alpha[k], beta[k]
    pr = np.zeros((128, 3 * N_FLOWS + 1), np.float32)
    comp = np.arange(128) // 32  # component index per partition
    for k in range(n_flows):
        dprev = -x0s[0] if k == 0 else x0s[k - 1] - x0s[k]
        pr[:, k] = dprev[comp]
        pr[:, N_FLOWS + 1 + k] = alpha[k]
        pr[:, 2 * N_FLOWS + 1 + k] = beta[k]
    pr[:, N_FLOWS] = x0s[n_flows - 1][comp]

    # stationaries: 0..3 reduce (comp partitions -> packed), 4..7 broadcast
    wm = np.zeros((8, 128, 128), np.float32)
    b = np.arange(NB)
    for j in range(4):
        for c in range(4):
            wm[j, 32 * c + b, 32 * j + b] = 1.0
            wm[4 + j, 32 * j + b, 32 * c + b] = 1.0
    return pr, wm


def kernel(X, x0s, alpha_primes, beta_primes):
    from concourse.bass_utils import run_bass_kernel_spmd

    X = np.asarray(X, np.float32)
    pr, wm = _host_params(x0s, alpha_primes, beta_primes)

    if "nc" not in _CACHE:
        _CACHE["nc"] = _build_program()
    nc = _CACHE["nc"]

    in_maps = []
    for c in range(N_CORES):
        shard = X[c * S:(c + 1) * S]
        in_maps.append({
            "xt": np.ascontiguousarray(shard.T),
            "wm": wm,
            "pr": pr,
        })
    res = run_bass_kernel_spmd(nc, in_maps, list(range(N_CORES)))
    out = np.empty((BATCH, DIM), np.float32)
    for c in range(N_CORES):
        out[c * S:(c + 1) * S] = res.results[c]["ot"].T
    return out


# revision 4
# speedup vs baseline: 2.5302x; 2.5302x over previous
"""Trainium2 Bass kernel for a chain of 20 radial flows on [8388608, 4] fp32.

Strategy: pure data parallel over 8 NeuronCores. Each core gets B/8 samples,
host-transposed to [4, S] so components sit on separate SBUF partitions
(partition 32*c + b holds component c of sample-block b).

Per flow k (sequential chain):
  d   = v + delta_{k-1}            (delta folded as per-partition ACT bias)
  sq  = d*d                        ACT Square (bf16 out)
  r2  = sum_c sq[c]                PE matmul with 0/1 bf16 stationary,
                                   output *packed* onto all 128 partitions
  r   = sqrt(r2)                   ACT
  w   = r + alpha_k                DVE tensor_scalar (per-partition AP)
  y   = 1/w                        DVE custom reciprocal (~51 ULP)
  m   = beta_k*y + 1               ACT Identity(scale=beta, bias=1) -> f32r
  m_b = broadcast m to comps       PE matmul with 0/1 stationary -> PSUM
  v  <- (v + delta_{k-1}) * m_b    DVE scalar_tensor_tensor (fused)
Final: out = v + x0[19].
"""

import sys

if "/opt/trn_rl_repo" not in sys.path:
    sys.path.insert(0, "/opt/trn_rl_repo")

from contextlib import ExitStack

import numpy as np

import concourse.bass as bass
import concourse.tile as tile
from concourse import bacc, mybir

F32 = mybir.dt.float32
F32R = mybir.dt.float32r
BF16 = mybir.dt.bfloat16

BATCH = 8388608
DIM = 4
N_FLOWS = 20
N_CORES = 8
S = BATCH // N_CORES          # samples per core
NB = 32                       # sample-blocks (per quadrant of partitions)
FC = 2048                     # chunk free-dim (columns) per tile
NT = S // NB // FC            # chunks per core
PK = FC // 4                  # packed (per-sample) free-dim per chunk

_CACHE = {}


def _build_program(n_flows=N_FLOWS, nt=NT):
    nc = bacc.Bacc("TRN2", target_bir_lowering=False, debug=False,
                   num_devices=N_CORES)
    s = nt * NB * FC
    xt = nc.dram_tensor("xt", [DIM, s], F32, kind="ExternalInput").ap()
    wr = nc.dram_tensor("wr", [4, 128, 128], BF16, kind="ExternalInput").ap()
    wb = nc.dram_tensor("wb", [4, 128, 128], F32R, kind="ExternalInput").ap()
    pr = nc.dram_tensor("pr", [128, 3 * N_FLOWS + 1], F32,
                        kind="ExternalInput").ap()
    ot = nc.dram_tensor("ot", [DIM, s], F32, kind="ExternalOutput").ap()

    # DRAM view: partition (c*32+b) <- comp c, block b; free (t, f)
    xt_r = xt.rearrange("c (b t f) -> (c b) t f", b=NB, f=FC)
    ot_r = ot.rearrange("c (b t f) -> (c b) t f", b=NB, f=FC)

    AL = mybir.AluOpType
    ACT = mybir.ActivationFunctionType

    def col(i):  # per-partition scalar AP from the params tile
        return pr_t[:, i:i + 1]

    with tile.TileContext(nc) as tc, ExitStack() as ctx:
        singles = ctx.enter_context(tc.tile_pool(name="singles", bufs=1))
        state = ctx.enter_context(tc.tile_pool(name="state", bufs=1))
        sq_pool = ctx.enter_context(tc.tile_pool(name="sq", bufs=2))
        pk_pool = ctx.enter_context(tc.tile_pool(name="pk", bufs=3))
        r2_pool = ctx.enter_context(
            tc.tile_pool(name="r2", bufs=2, space="PSUM"))
        mb_pool = ctx.enter_context(
            tc.tile_pool(name="mb", bufs=2, space="PSUM"))

        pr_t = singles.tile([128, 3 * N_FLOWS + 1], F32)
        nc.sync.dma_start(pr_t[:], pr[:])
        wr_t = singles.tile([128, 4, 128], BF16)
        nc.sync.dma_start(wr_t[:], wr.rearrange("j k m -> k j m"))
        wb_t = singles.tile([128, 4, 128], F32R)
        nc.sync.dma_start(wb_t[:], wb.rearrange("j k m -> k j m"))

        v = state.tile([128, nt * FC], F32)
        for t in range(nt):
            nc.sync.dma_start(v[:, bass.ts(t, FC)], xt_r[:, t, :])

        for k in range(n_flows):
            c_dprev, c_alpha, c_beta = k, N_FLOWS + 1 + k, 2 * N_FLOWS + 1 + k
            for t in range(nt):
                vchunk = v[:, bass.ts(t, FC)]
                sq = sq_pool.tile([128, FC], BF16)
                nc.scalar.activation(sq[:], vchunk, ACT.Square,
                                     bias=col(c_dprev), scale=1.0)
                r2 = r2_pool.tile([128, PK], F32)
                for j in range(4):
                    nc.tensor.matmul(
                        out=r2[:],
                        lhsT=wr_t[:, j, :],
                        rhs=sq[:, bass.ts(j, PK)],
                        start=(j == 0), stop=(j == 3))
                r = pk_pool.tile([128, PK], F32, tag="r")
                nc.scalar.activation(r[:], r2[:], ACT.Sqrt)
                w = pk_pool.tile([128, PK], F32, tag="w")
                nc.vector.tensor_scalar(w[:], r[:], col(c_alpha), None, AL.add)
                y = pk_pool.tile([128, PK], F32, tag="y")
                nc.vector.reciprocal_approx_fast(y[:], w[:])
                m = pk_pool.tile([128, PK], F32R, tag="m")
                nc.scalar.activation(m[:], y[:], ACT.Identity,
                                     bias=1.0, scale=col(c_beta))
                for h in range(2):  # two PSUM half-tiles so bcast/stt overlap
                    mb = mb_pool.tile([128, FC // 2], F32)
                    for jj in range(2):
                        j = 2 * h + jj
                        nc.tensor.matmul(
                            out=mb[:, bass.ts(jj, PK)],
                            lhsT=wb_t[:, j, :],
                            rhs=m[:], start=True, stop=True)
                    vh = v[:, t * FC + h * (FC // 2):
                           t * FC + (h + 1) * (FC // 2)]
                    nc.vector.scalar_tensor_tensor(
                        out=vh, in0=vh, scalar=col(c_dprev), in1=mb[:],
                        op0=AL.add, op1=AL.mult)

        for t in range(nt):
            vchunk = v[:, bass.ts(t, FC)]
            nc.vector.tensor_scalar(vchunk, vchunk, col(N_FLOWS), None, AL.add)
            nc.sync.dma_start(ot_r[:, t, :], vchunk)

    nc.compile()
    return nc


def _host_params(x0s, alpha_primes, beta_primes, n_flows=N_FLOWS):
    x0s = np.asarray(x0s, np.float32)
    sp_a = np.logaddexp(np.float32(0.0), np.asarray(alpha_primes, np.float32))
    sp_b = np.logaddexp(np.float32(0.0), np.asarray(beta_primes, np.float32))
    alpha = sp_a.astype(np.float32)
    beta = (-alpha + sp_b).astype(np.float32)

    # params: [128, 3*N_FLOWS+1]: dprev[k], final, alpha[k], beta[k]
    pr = np.zeros((128, 3 * N_FLOWS + 1), np.float32)
    comp = np.arange(128) // 32  # component index per partition
    for k in range(n_flows):
        dprev = -x0s[0] if k == 0 else x0s[k - 1] - x0s[k]
        pr[:, k] = dprev[comp]
        pr[:, N_FLOWS + 1 + k] = alpha[k]
        pr[:, 2 * N_FLOWS + 1 + k] = beta[k]
    pr[:, N_FLOWS] = x0s[n_flows - 1][comp]

    # stationaries: wr reduce (comp partitions -> packed), wb broadcast
    import ml_dtypes
    wr = np.zeros((4, 128, 128), np.float32)
    wb = np.zeros((4, 128, 128), np.float32)
    b = np.arange(NB)
    for j in range(4):
        for c in range(4):
            wr[j, 32 * c + b, 32 * j + b] = 1.0
            wb[j, 32 * j + b, 32 * c + b] = 1.0
    return pr, wr.astype(ml_dtypes.bfloat16), wb


def kernel(X, x0s, alpha_primes, beta_primes):
    from concourse.bass_utils import run_bass_kernel_spmd

    X = np.asarray(X, np.float32)
    pr, wr, wb = _host_params(x0s, alpha_primes, beta_primes)

    if "nc" not in _CACHE:
        _CACHE["nc"] = _build_program()
    nc = _CACHE["nc"]

    in_maps = []
    for c in range(N_CORES):
        shard = X[c * S:(c + 1) * S]
        in_maps.append({
            "xt": np.ascontiguousarray(shard.T),
            "wr": wr,
            "wb": wb,
            "pr": pr,
        })
    res = run_bass_kernel_spmd(nc, in_maps, list(range(N_CORES)))
    out = np.empty((BATCH, DIM), np.float32)
    for c in range(N_CORES):
        out[c * S:(c + 1) * S] = res.results[c]["ot"].T
    return out


# revision 8
# speedup vs baseline: 2.8349x; 1.1204x over previous
"""Trainium2 Bass kernel for a chain of 20 radial flows on [8388608, 4] fp32.

Strategy: pure data parallel over 8 NeuronCores. Each core gets B/8 samples,
host-transposed to [4, S] so components sit on separate SBUF partitions
(partition 32*c + b holds component c of sample-block b).

Per flow k (sequential chain):
  d   = v + delta_{k-1}            (delta folded as per-partition ACT bias)
  sq  = d*d                        ACT Square (bf16 out)
  r2  = sum_c sq[c]                PE matmul with 0/1 bf16 stationary,
                                   output *packed* onto all 128 partitions
  r   = sqrt(r2)                   ACT
  w   = r + alpha_k                DVE tensor_scalar (per-partition AP)
  y   = 1/w                        DVE custom reciprocal (~51 ULP)
  m   = beta_k*y + 1               ACT Identity(scale=beta, bias=1) -> f32r
  m_b = broadcast m to comps       PE matmul with 0/1 stationary -> PSUM
  v  <- (v + delta_{k-1}) * m_b    DVE scalar_tensor_tensor (fused)
Final: out = v + x0[19].
"""

import sys

if "/opt/trn_rl_repo" not in sys.path:
    sys.path.insert(0, "/opt/trn_rl_repo")

from contextlib import ExitStack

import numpy as np

import concourse.bass as bass
import concourse.tile as tile
from concourse import bacc, mybir

F32 = mybir.dt.float32
F32R = mybir.dt.float32r
BF16 = mybir.dt.bfloat16

BATCH = 8388608
DIM = 4
N_FLOWS = 20
N_CORES = 8
S = BATCH // N_CORES          # samples per core
NB = 32                       # sample-blocks (per quadrant of partitions)
FC = 2048                     # chunk free-dim (columns) per tile
NT = S // NB // FC            # chunks per core
PK = FC // 4                  # packed (per-sample) free-dim per chunk

_CACHE = {}


def _build_program(n_flows=N_FLOWS, nt=NT):
    nc = bacc.Bacc("TRN2", target_bir_lowering=False, debug=False,
                   num_devices=N_CORES)
    s = nt * NB * FC
    xt = nc.dram_tensor("xt", [DIM, s], F32, kind="ExternalInput").ap()
    wr = nc.dram_tensor("wr", [4, 128, 128], BF16, kind="ExternalInput").ap()
    wb = nc.dram_tensor("wb", [4, 128, 128], F32R, kind="ExternalInput").ap()
    pr = nc.dram_tensor("pr", [128, 3 * N_FLOWS + 1], F32,
                        kind="ExternalInput").ap()
    ot = nc.dram_tensor("ot", [DIM, s], F32, kind="ExternalOutput").ap()

    # DRAM view: partition (c*32+b) <- comp c, block b; free (t, f)
    xt_r = xt.rearrange("c (b t f) -> (c b) t f", b=NB, f=FC)
    ot_r = ot.rearrange("c (b t f) -> (c b) t f", b=NB, f=FC)

    AL = mybir.AluOpType
    ACT = mybir.ActivationFunctionType

    def col(i):  # per-partition scalar AP from the params tile
        return pr_t[:, i:i + 1]

    with tile.TileContext(nc) as tc, ExitStack() as ctx:
        singles = ctx.enter_context(tc.tile_pool(name="singles", bufs=1))
        state = ctx.enter_context(tc.tile_pool(name="state", bufs=1))
        sq_pool = ctx.enter_context(tc.tile_pool(name="sq", bufs=3))
        pk_pool = ctx.enter_context(tc.tile_pool(name="pk", bufs=4))
        r2_pool = ctx.enter_context(
            tc.tile_pool(name="r2", bufs=2, space="PSUM"))
        mb_pool = ctx.enter_context(
            tc.tile_pool(name="mb", bufs=3, space="PSUM"))

        pr_t = singles.tile([128, 3 * N_FLOWS + 1], F32)
        nc.sync.dma_start(pr_t[:], pr[:])
        wr_t = singles.tile([128, 4, 128], BF16)
        nc.sync.dma_start(wr_t[:], wr.rearrange("j k m -> k j m"))
        wb_t = singles.tile([128, 4, 128], F32R)
        nc.sync.dma_start(wb_t[:], wb.rearrange("j k m -> k j m"))

        v = state.tile([128, nt * FC], F32)
        for t in range(nt):
            nc.sync.dma_start(v[:, bass.ts(t, FC)], xt_r[:, t, :])

        for k in range(n_flows):
            c_dprev, c_alpha, c_beta = k, N_FLOWS + 1 + k, 2 * N_FLOWS + 1 + k
            for t in range(nt):
                vchunk = v[:, bass.ts(t, FC)]
                sq = sq_pool.tile([128, FC], BF16)
                nc.scalar.activation(sq[:], vchunk, ACT.Square,
                                     bias=col(c_dprev), scale=1.0)
                r2 = r2_pool.tile([128, PK], F32)
                for j in range(4):
                    nc.tensor.matmul(
                        out=r2[:],
                        lhsT=wr_t[:, j, :],
                        rhs=sq[:, bass.ts(j, PK)],
                        start=(j == 0), stop=(j == 3))
                r = pk_pool.tile([128, PK], F32, tag="r")
                nc.scalar.activation(r[:], r2[:], ACT.Sqrt)
                w = pk_pool.tile([128, PK], F32, tag="w")
                nc.vector.tensor_scalar(w[:], r[:], col(c_alpha), None, AL.add)
                y = pk_pool.tile([128, PK], F32, tag="y")
                nc.vector.reciprocal_approx_fast(y[:], w[:])
                m = pk_pool.tile([128, PK], F32R, tag="m")
                nc.scalar.activation(m[:], y[:], ACT.Identity,
                                     bias=1.0, scale=col(c_beta))
                for h in range(2):  # two PSUM half-tiles so bcast/stt overlap
                    mb = mb_pool.tile([128, FC // 2], F32)
                    for jj in range(2):
                        j = 2 * h + jj
                        nc.tensor.matmul(
                            out=mb[:, bass.ts(jj, PK)],
                            lhsT=wb_t[:, j, :],
                            rhs=m[:], start=True, stop=True)
                    vh = v[:, t * FC + h * (FC // 2):
                           t * FC + (h + 1) * (FC // 2)]
                    nc.vector.scalar_tensor_tensor(
                        out=vh, in0=vh, scalar=col(c_dprev), in1=mb[:],
                        op0=AL.add, op1=AL.mult)

        for t in range(nt):
            vchunk = v[:, bass.ts(t, FC)]
            nc.vector.tensor_scalar(vchunk, vchunk, col(N_FLOWS), None, AL.add)
            nc.sync.dma_start(ot_r[:, t, :], vchunk)

    nc.compile()
    return nc


def _host_params(x0s, alpha_primes, beta_primes, n_flows=N_FLOWS):
    x0s = np.asarray(x0s, np.float32)
    sp_a = np.logaddexp(np.float32(0.0), np.asarray(alpha_primes, np.float32))
    sp_b = np.logaddexp(np.float32(0.0), np.asarray(beta_primes, np.float32))
    alpha = sp_a.astype(np.float32)
    beta = (-alpha + sp_b).astype(np.float32)

    # params: [128, 3*N_FLOWS+1]: dprev[k], final, alpha[k], beta[k]
    pr = np.zeros((128, 3 * N_FLOWS + 1), np.float32)
    comp = np.arange(128) // 32  # component index per partition
    for k in range(n_flows):
        dprev = -x0s[0] if k == 0 else x0s[k - 1] - x0s[k]
        pr[:, k] = dprev[comp]
        pr[:, N_FLOWS + 1 + k] = alpha[k]
        pr[:, 2 * N_FLOWS + 1 + k] = beta[k]
    pr[:, N_FLOWS] = x0s[n_flows - 1][comp]

    # stationaries: wr reduce (comp partitions -> packed), wb broadcast (K=32)
    import ml_dtypes
    wr = np.zeros((4, 128, 128), np.float32)
    wb = np.zeros((4, 128, 128), np.float32)
    b = np.arange(NB)
    for j in range(4):
        for c in range(4):
            wr[j, 32 * c + b, 32 * j + b] = 1.0
            wb[j, 32 * j + b, 32 * c + b] = 1.0
    return pr, wr.astype(ml_dtypes.bfloat16), wb


def kernel(X, x0s, alpha_primes, beta_primes):
    from concourse.bass_utils import run_bass_kernel_spmd

    X = np.asarray(X, np.float32)
    pr, wr, wb = _host_params(x0s, alpha_primes, beta_primes)

    if "nc" not in _CACHE:
        _CACHE["nc"] = _build_program()
    nc = _CACHE["nc"]

    in_maps = []
    for c in range(N_CORES):
        shard = X[c * S:(c + 1) * S]
        in_maps.append({
            "xt": np.ascontiguousarray(shard.T),
            "wr": wr,
            "wb": wb,
            "pr": pr,
        })
    res = run_bass_kernel_spmd(nc, in_maps, list(range(N_CORES)))
    out = np.empty((BATCH, DIM), np.float32)
    for c in range(N_CORES):
        out[c * S:(c + 1) * S] = res.results[c]["ot"].T
    return out


# revision 9
# speedup vs baseline: 2.8619x; 1.0095x over previous
"""Trainium2 Bass kernel for a chain of 20 radial flows on [8388608, 4] fp32.

Strategy: pure data parallel over 8 NeuronCores. Each core gets B/8 samples,
host-transposed to [4, S] so components sit on separate SBUF partitions
(partition 32*c + b holds component c of sample-block b).

Per flow k (sequential chain):
  d   = v + delta_{k-1}            (delta folded as per-partition ACT bias)
  sq  = d*d                        ACT Square (bf16 out)
  r2  = sum_c sq[c]                PE matmul with 0/1 bf16 stationary,
                                   output *packed* onto all 128 partitions
  r   = sqrt(r2)                   ACT
  w   = r + alpha_k                DVE tensor_scalar (per-partition AP)
  y   = 1/w                        DVE custom reciprocal (~51 ULP)
  m   = beta_k*y + 1               ACT Identity(scale=beta, bias=1) -> f32r
  m_b = broadcast m to comps       PE matmul with 0/1 stationary -> PSUM
  v  <- (v + delta_{k-1}) * m_b    DVE scalar_tensor_tensor (fused)
Final: out = v + x0[19].
"""

import sys

if "/opt/trn_rl_repo" not in sys.path:
    sys.path.insert(0, "/opt/trn_rl_repo")

from contextlib import ExitStack

import numpy as np

import concourse.bass as bass
import concourse.tile as tile
from concourse import bacc, mybir

F32 = mybir.dt.float32
F32R = mybir.dt.float32r
BF16 = mybir.dt.bfloat16

BATCH = 8388608
DIM = 4
N_FLOWS = 20
N_CORES = 8
S = BATCH // N_CORES          # samples per core
NB = 32                       # sample-blocks (per quadrant of partitions)
FC = 2048                     # chunk free-dim (columns) per tile
NT = S // NB // FC            # chunks per core
PK = FC // 4                  # packed (per-sample) free-dim per chunk

_CACHE = {}


def _build_program(n_flows=N_FLOWS, nt=NT):
    nc = bacc.Bacc("TRN2", target_bir_lowering=False, debug=False,
                   num_devices=N_CORES)
    s = nt * NB * FC
    xt = nc.dram_tensor("xt", [DIM, s], F32, kind="ExternalInput").ap()
    wr = nc.dram_tensor("wr", [4, 128, 128], BF16, kind="ExternalInput").ap()
    wb = nc.dram_tensor("wb", [4, 128, 128], F32R, kind="ExternalInput").ap()
    pr = nc.dram_tensor("pr", [128, 3 * N_FLOWS + 1], F32,
                        kind="ExternalInput").ap()
    ot = nc.dram_tensor("ot", [DIM, s], F32, kind="ExternalOutput").ap()

    # DRAM view: partition (c*32+b) <- comp c, block b; free (t, f)
    xt_r = xt.rearrange("c (b t f) -> (c b) t f", b=NB, f=FC)
    ot_r = ot.rearrange("c (b t f) -> (c b) t f", b=NB, f=FC)

    AL = mybir.AluOpType
    ACT = mybir.ActivationFunctionType

    def col(i):  # per-partition scalar AP from the params tile
        return pr_t[:, i:i + 1]

    with tile.TileContext(nc) as tc, ExitStack() as ctx:
        singles = ctx.enter_context(tc.tile_pool(name="singles", bufs=1))
        state = ctx.enter_context(tc.tile_pool(name="state", bufs=1))
        sq_pool = ctx.enter_context(tc.tile_pool(name="sq", bufs=4))
        pk_pool = ctx.enter_context(tc.tile_pool(name="pk", bufs=6))
        r2_pool = ctx.enter_context(
            tc.tile_pool(name="r2", bufs=2, space="PSUM"))
        mb_pool = ctx.enter_context(
            tc.tile_pool(name="mb", bufs=3, space="PSUM"))

        pr_t = singles.tile([128, 3 * N_FLOWS + 1], F32)
        nc.sync.dma_start(pr_t[:], pr[:])
        wr_t = singles.tile([128, 4, 128], BF16)
        nc.sync.dma_start(wr_t[:], wr.rearrange("j k m -> k j m"))
        wb_t = singles.tile([128, 4, 128], F32R)
        nc.sync.dma_start(wb_t[:], wb.rearrange("j k m -> k j m"))

        v = state.tile([128, nt * FC], F32)
        for t in range(nt):
            nc.sync.dma_start(v[:, bass.ts(t, FC)], xt_r[:, t, :])

        for k in range(n_flows):
            c_dprev, c_alpha, c_beta = k, N_FLOWS + 1 + k, 2 * N_FLOWS + 1 + k
            for t in range(nt):
                vchunk = v[:, bass.ts(t, FC)]
                sq = sq_pool.tile([128, FC], BF16)
                nc.scalar.activation(sq[:], vchunk, ACT.Square,
                                     bias=col(c_dprev), scale=1.0)
                r2 = r2_pool.tile([128, PK], F32)
                for j in range(4):
                    nc.tensor.matmul(
                        out=r2[:],
                        lhsT=wr_t[:, j, :],
                        rhs=sq[:, bass.ts(j, PK)],
                        start=(j == 0), stop=(j == 3))
                r = pk_pool.tile([128, PK], F32, tag="r")
                nc.scalar.activation(r[:], r2[:], ACT.Sqrt)
                w = pk_pool.tile([128, PK], F32, tag="w")
                nc.vector.tensor_scalar(w[:], r[:], col(c_alpha), None, AL.add)
                y = pk_pool.tile([128, PK], F32, tag="y")
                nc.vector.reciprocal_approx_fast(y[:], w[:])
                m = pk_pool.tile([128, PK], F32R, tag="m")
                nc.scalar.activation(m[:], y[:], ACT.Identity,
                                     bias=1.0, scale=col(c_beta))
                for h in range(2):  # two PSUM half-tiles so bcast/stt overlap
                    mb = mb_pool.tile([128, FC // 2], F32)
                    for jj in range(2):
                        j = 2 * h + jj
                        nc.tensor.matmul(
                            out=mb[:, bass.ts(jj, PK)],
                            lhsT=wb_t[:, j, :],
                            rhs=m[:], start=True, stop=True)
                    vh = v[:, t * FC + h * (FC // 2):
                           t * FC + (h + 1) * (FC // 2)]
                    nc.vector.scalar_tensor_tensor(
                        out=vh, in0=vh, scalar=col(c_dprev), in1=mb[:],
                        op0=AL.add, op1=AL.mult)

        for t in range(nt):
            vchunk = v[:, bass.ts(t, FC)]
            nc.vector.tensor_scalar(vchunk, vchunk, col(N_FLOWS), None, AL.add)
            nc.sync.dma_start(ot_r[:, t, :], vchunk)

    nc.compile()
    return nc


def _host_params(x0s, alpha_primes, beta_primes, n_flows=N_FLOWS):
    x0s = np.asarray(x0s, np.float32)
    sp_a = np.logaddexp(np.float32(0.0), np.asarray(alpha_primes, np.float32))
    sp_b = np.logaddexp(np.float32(0.0), np.asarray(beta_primes, np.float32))
    alpha = sp_a.astype(np.float32)
    beta = (-alpha + sp_b).astype(np.float32)

    # params: [128, 3*N_FLOWS+1]: dprev[k], final, alpha[k], beta[k]
    pr = np.zeros((128, 3 * N_FLOWS + 1), np.float32)
    comp = np.arange(128) // 32  # component index per partition
    for k in range(n_flows):
        dprev = -x0s[0] if k == 0 else x0s[k - 1] - x0s[k]
        pr[:, k] = dprev[comp]
        pr[:, N_FLOWS + 1 + k] = alpha[k]
        pr[:, 2 * N_FLOWS + 1 + k] = beta[k]
    pr[:, N_FLOWS] = x0s[n_flows - 1][comp]

    # stationaries: wr reduce (comp partitions -> packed), wb broadcast (K=32)
    import ml_dtypes
    wr = np.zeros((4, 128, 128), np.float32)
    wb = np.zeros((4, 128, 128), np.float32)
    b = np.arange(NB)
    for j in range(4):
        for c in range(4):
            wr[j, 32 * c + b, 32 * j + b] = 1.0
            wb[j, 32 * j + b, 32 * c + b] = 1.0
    return pr, wr.astype(ml_dtypes.bfloat16), wb


def kernel(X, x0s, alpha_primes, beta_primes):
    from concourse.bass_utils import run_bass_kernel_spmd

    X = np.asarray(X, np.float32)
    pr, wr, wb = _host_params(x0s, alpha_primes, beta_primes)

    if "nc" not in _CACHE:
        _CACHE["nc"] = _build_program()
    nc = _CACHE["nc"]

    in_maps = []
    for c in range(N_CORES):
        shard = X[c * S:(c + 1) * S]
        in_maps.append({
            "xt": np.ascontiguousarray(shard.T),
            "wr": wr,
            "wb": wb,
            "pr": pr,
        })
    res = run_bass_kernel_spmd(nc, in_maps, list(range(N_CORES)))
    out = np.empty((BATCH, DIM), np.float32)
    for c in range(N_CORES):
        out[c * S:(c + 1) * S] = res.results[c]["ot"].T
    return out
